# revision 27
# baseline (speedup 1.0000x reference)
"""MiniMax-M2 decoder layer on 8 trn2 NeuronCores (v2).

Sharding: fully sequence-sharded. Each core owns 512 tokens of the
flattened (B*S)=4096 token stream and recomputes the 512-token KV halo
locally (no collectives in the attention block). The MLP is
data-parallel over tokens too: every core streams the full bf16
w1/w3/w2 from its DRAM once (~100MB, hidden under the 655us of MLP
matmul) while computing only its own 512 tokens -> ZERO collectives in
the whole kernel.

All weights are pre-cast to bf16 and pre-packed on the host so every
weight DMA is a large contiguous block (128KB-512KB, 1-4KB per
partition line).

Matmuls run in bf16 against f32 psum. Softmax skips the
max-subtraction (scores are bounded ~6 for this layer's magnitudes).

Self-contained: includes the BIR wait-splitting fix this container's
walrus build needs (1 semaphore wait per instruction max).
"""

import json
import sys
import types

import numpy as np

import concourse.bass as bass
import concourse.mybir as mybir
import concourse.tile as tile
from concourse.masks import make_identity

# ---------------------------------------------------------------- constants
B, S, HID = 2, 2048, 2048
H, HK, D = 16, 4, 128
RD = 64
IM = 8192
WIN = 512
EPS = 1e-6
THETA = 10000.0
SCALE = D ** -0.5

NCORES = 8
TOK = 512              # own tokens per core
EXT = 1024             # halo + own
NEG = -1e9

F32 = mybir.dt.float32
F32R = mybir.dt.float32r
BF16 = mybir.dt.bfloat16
AF = mybir.ActivationFunctionType

KT = 8                 # 128-wide key tiles over EXT
NM = HID // 128        # 16 hid tiles
CW = 512               # im chunk width for w1/w3 streaming
NCH1 = IM // CW        # 16 chunks
NMI = IM // 128        # 64 im tiles
HS = 4                 # hid column quarters for w2 streaming
HSW = HID // HS        # 512

# ------------------------------------------------------- walrus wait-split fix
MAX_WAITS = 1


def _split_excess_waits(bir_bytes: bytes) -> bytes:
    m = json.loads(bir_bytes)
    ctr = [0]

    def fix_insts(insts):
        out = []
        for ins in insts:
            si = ins.get("sync_info")
            ow = (si or {}).get("on_wait") or []
            if len(ow) > MAX_WAITS:
                eng = ins["engine"]
                keep = ow[-MAX_WAITS:]
                excess = ow[:-MAX_WAITS]
                ins["sync_info"]["on_wait"] = keep
                for i in range(0, len(excess), MAX_WAITS):
                    ctr[0] += 1
                    out.append({
                        "debug": ins.get("debug", 0),
                        "engine": eng,
                        "ins": [],
                        "name": f"I-waitfix-{ctr[0]}",
                        "opcode": "NoOp",
                        "outs": [],
                        "sync_info": {"on_update": [],
                                      "on_wait": excess[i:i + MAX_WAITS]},
                        "text_hint": "waitfix",
                    })
            out.append(ins)
        return out

    def walk(o):
        if isinstance(o, dict):
            if isinstance(o.get("instructions"), list):
                o["instructions"] = fix_insts(o["instructions"])
            for v in o.values():
                walk(v)
        elif isinstance(o, list):
            for v in o:
                walk(v)

    walk(m)
    return json.dumps(m).encode()


class _BassFixed(bass.Bass):
    def to_json_bytes(self) -> bytes:
        return _split_excess_waits(super().to_json_bytes())


def _register_ntff_hook():
    """Provide antenv.axon_hooks (missing in this image) so trace=True works."""
    if "antenv.axon_hooks" in sys.modules:
        return
    try:
        import trn_agent_boot.trn_boot as tb
    except ImportError:
        return
    mod = types.ModuleType("antenv.axon_hooks")
    holder = [None]
    mod.set_axon_ntff_profile_hook = lambda h: holder.__setitem__(0, h)
    mod.get_axon_ntff_profile_hook = lambda: holder[0]
    sys.modules["antenv.axon_hooks"] = mod
    try:
        mod.set_axon_ntff_profile_hook(
            tb._ntff_profile_via_ctypes("/opt/axon/libaxon_pjrt.so"))
    except Exception:
        pass


# ---------------------------------------------------------------- the program
def build_nc():
    nc = _BassFixed(num_devices=NCORES, target_bir_lowering=False)

    xT = nc.dram_tensor("xT", [HID, EXT], F32R, kind="ExternalInput")
    wqT = nc.dram_tensor("wqT", [HID, H * D], BF16, kind="ExternalInput")
    wkT = nc.dram_tensor("wkT", [HID, HK * D], BF16, kind="ExternalInput")
    wvT = nc.dram_tensor("wvT", [HID, HK * D], BF16, kind="ExternalInput")
    woT = nc.dram_tensor("woT", [H * D, HID], BF16, kind="ExternalInput")
    w1P = nc.dram_tensor("w1P", [NCH1, NM, 128, CW], BF16,
                         kind="ExternalInput")
    w3P = nc.dram_tensor("w3P", [NCH1, NM, 128, CW], BF16,
                         kind="ExternalInput")
    w2P = nc.dram_tensor("w2P", [HS, NMI, 128, HSW], BF16,
                         kind="ExternalInput")
    ln1w = nc.dram_tensor("ln1w", [128, 16], F32, kind="ExternalInput")
    ln2w = nc.dram_tensor("ln2w", [128, 16], F32, kind="ExternalInput")
    qnw = nc.dram_tensor("qnw", [128, 16], F32, kind="ExternalInput")
    knw = nc.dram_tensor("knw", [128, 4], F32, kind="ExternalInput")
    cos_q = nc.dram_tensor("cos_q", [RD, TOK], F32, kind="ExternalInput")
    sinS_q = nc.dram_tensor("sinS_q", [RD, TOK], F32, kind="ExternalInput")
    cos_k = nc.dram_tensor("cos_k", [RD, EXT], F32, kind="ExternalInput")
    sinS_k = nc.dram_tensor("sinS_k", [RD, EXT], F32, kind="ExternalInput")
    halo = nc.dram_tensor("halo", [128, 8], F32, kind="ExternalInput")
    band = nc.dram_tensor("band", [128, 1408], BF16, kind="ExternalInput")
    ones_c = nc.dram_tensor("ones_c", [128, 1], F32R, kind="ExternalInput")
    ones_r = nc.dram_tensor("ones_r", [1, 128], F32R, kind="ExternalInput")
    ones_b = nc.dram_tensor("ones_b", [128, 1], BF16, kind="ExternalInput")

    out = nc.dram_tensor("out", [TOK, HID], F32, kind="ExternalOutput")

    with tile.TileContext(nc) as tc:
        with tc.tile_pool(name="consts", bufs=1) as cst, \
             tc.tile_pool(name="smalls", bufs=2) as sml, \
             tc.tile_pool(name="h2keep", bufs=1) as h2keep, \
             tc.tile_pool(name="dram", bufs=1, space="DRAM") as dram:

            # ---------------- constants
            onec = cst.tile([128, 1], F32R)
            nc.sync.dma_start(onec[:], ones_c[:])
            oner = cst.tile([1, 128], F32R)
            nc.sync.dma_start(oner[:], ones_r[:])
            oneb = cst.tile([128, 1], BF16)
            nc.sync.dma_start(oneb[:], ones_b[:])
            ln1w_s = cst.tile([128, 16], F32)
            nc.sync.dma_start(ln1w_s[:], ln1w[:])
            ln2w_s = cst.tile([128, 16], F32)
            nc.sync.dma_start(ln2w_s[:], ln2w[:])
            qnw_s = cst.tile([128, 16], F32)
            nc.sync.dma_start(qnw_s[:], qnw[:])
            knw_s = cst.tile([128, 4], F32)
            nc.sync.dma_start(knw_s[:], knw[:])
            eps_s = cst.tile([1, 1], F32)
            nc.vector.memset(eps_s[:], EPS)
            identF = cst.tile([128, 128], F32)
            make_identity(nc, identF[:])
            identR = cst.tile([128, 128], F32R)
            nc.vector.tensor_copy(identR[:], identF[:])

            h2s = h2keep.tile([128, NM, TOK], BF16)
            x2tok = dram.tile([TOK, HID], F32)

            with tc.tile_pool(name="rowps", bufs=3, space="PSUM") as rps, \
                 tc.tile_pool(name="bigps", bufs=3, space="PSUM") as bps, \
                 tc.tile_pool(name="bcps", bufs=1, space="PSUM") as bcps:

                def bcast_row(row_f32r, width, tag):
                    """[1,width] F32R row -> [128,width] F32 psum broadcast."""
                    p = bcps.tile([128, width], F32, tag="bc")
                    for j in range(0, width, 512):
                        w = min(512, width - j)
                        nc.tensor.matmul(p[:, j:j + w], oner[:],
                                         row_f32r[:, j:j + w],
                                         start=True, stop=True)
                    return p

                def row_rsqrt_bcast(acc_parts, width, denom, tag):
                    """psum [1,*] sum-of-squares parts -> [128,width] psum with
                    1/sqrt(acc/denom + eps) broadcast over partitions."""
                    srow = sml.tile([1, width], F32, tag="srow")
                    for (p, j, w) in acc_parts:
                        nc.scalar.activation(out=srow[:, j:j + w], in_=p[:],
                                             func=AF.Sqrt, bias=eps_s[:],
                                             scale=1.0 / denom)
                    rrow = sml.tile([1, width], F32R, tag="rrow")
                    with nc.allow_low_precision(reason="f32r rounding ok"):
                        nc.vector.reciprocal(rrow[:], srow[:])
                    return bcast_row(rrow, width, tag)

                # ======== q/k/v activations
                with tc.tile_pool(name="qkv", bufs=1) as qkv:
                    qT = qkv.tile([128, H, TOK], BF16)
                    kT = qkv.tile([128, HK, EXT], BF16)
                    Vb = qkv.tile([128, KT, HK * D], BF16)

                    # ============ phase A+B: ln1-RMS + Q/K/V projections
                    with tc.tile_pool(name="hp", bufs=1) as hp, \
                         tc.tile_pool(name="xs", bufs=4) as xs, \
                         tc.tile_pool(name="sqp", bufs=2) as sqp, \
                         tc.tile_pool(name="wat", bufs=1) as wat:
                        vT = hp.tile([128, HK, EXT], F32R, tag="vT")
                        hTh = hp.tile([128, NM, EXT], BF16, tag="h")

                        # resident attention weights (big contiguous strips)
                        wks = wat.tile([128, NM, HK * D], BF16)
                        wvs = wat.tile([128, NM, HK * D], BF16)
                        nc.sync.dma_start(
                            wks[:], wkT[:].rearrange("(i p) j -> p i j", p=128))
                        nc.sync.dma_start(
                            wvs[:], wvT[:].rearrange("(i p) j -> p i j", p=128))

                        for half in (0, 1):
                            c0 = half * 512
                            acc = rps.tile([1, 512], F32, tag="row")
                            for i in range(NM):
                                xt = xs.tile([128, 512], F32R, tag="xt")
                                nc.scalar.dma_start(
                                    xt[:],
                                    xT[i * 128:(i + 1) * 128, c0:c0 + 512])
                                sq = sqp.tile([128, 512], F32R, tag="sq")
                                nc.vector.tensor_mul(sq[:], xt[:], xt[:])
                                nc.tensor.matmul(acc[:], onec[:], sq[:],
                                                 start=(i == 0),
                                                 stop=(i == NM - 1))
                            s1b = row_rsqrt_bcast([(acc, 0, 512)], 512,
                                                  HID, "l1")
                            for i in range(NM):
                                xt = xs.tile([128, 512], F32R, tag="xt")
                                nc.scalar.dma_start(
                                    xt[:],
                                    xT[i * 128:(i + 1) * 128, c0:c0 + 512])
                                tmp = sqp.tile([128, 512], F32R, tag="sq")
                                nc.vector.tensor_mul(tmp[:], xt[:], s1b[:])
                                nc.vector.tensor_scalar_mul(
                                    hTh[:, i, c0:c0 + 512], tmp[:],
                                    ln1w_s[:, i:i + 1])

                            # K / V for this half
                            for m in range(HK):
                                pk = bps.tile([128, 512], F32, tag="big")
                                for i in range(NM):
                                    nc.tensor.matmul(
                                        pk[:], wks[:, i, m * 128:(m + 1) * 128],
                                        hTh[:, i, c0:c0 + 512],
                                        start=(i == 0), stop=(i == NM - 1))
                                pv = bps.tile([128, 512], F32, tag="big")
                                for i in range(NM):
                                    nc.tensor.matmul(
                                        pv[:], wvs[:, i, m * 128:(m + 1) * 128],
                                        hTh[:, i, c0:c0 + 512],
                                        start=(i == 0), stop=(i == NM - 1))
                                nc.vector.tensor_copy(kT[:, m, c0:c0 + 512],
                                                      pk[:])
                                nc.vector.tensor_copy(vT[:, m, c0:c0 + 512],
                                                      pv[:])

                        # Q projection (own tokens only), wq in two halves
                        for qh in range(2):
                            wqs = wat.tile([128, NM, H * D // 2], BF16,
                                           tag="wq", bufs=1,
                                           name=f"wqs{qh}")
                            nc.sync.dma_start(
                                wqs[:],
                                wqT[:, qh * 1024:(qh + 1) * 1024]
                                .rearrange("(i p) j -> p i j", p=128))
                            for mm in range(H // 2):
                                m = qh * 8 + mm
                                pq = bps.tile([128, 512], F32, tag="big")
                                for i in range(NM):
                                    nc.tensor.matmul(
                                        pq[:],
                                        wqs[:, i, mm * 128:(mm + 1) * 128],
                                        hTh[:, i, 512:1024],
                                        start=(i == 0), stop=(i == NM - 1))
                                nc.vector.tensor_copy(qT[:, m, :], pq[:])

                        # transpose vT -> token-major bf16 Vb
                        for kt in range(KT):
                            for g in range(HK):
                                pt = bps.tile([128, 128], F32R, tag="big")
                                nc.tensor.transpose(
                                    pt[:], vT[:, g, kt * 128:(kt + 1) * 128],
                                    identR[:])
                                nc.vector.tensor_copy(
                                    Vb[:, kt, g * 128:(g + 1) * 128], pt[:])

                    # ---------------- fused q/k RMSNorm + partial RoPE
                    with tc.tile_pool(name="nrm", bufs=1) as nrm:
                        cq_s = nrm.tile([RD, TOK], F32)
                        nc.sync.dma_start(cq_s[:], cos_q[:])
                        sq_s = nrm.tile([RD, TOK], F32)
                        nc.sync.dma_start(sq_s[:], sinS_q[:])
                        ck_s = nrm.tile([RD, EXT], F32)
                        nc.sync.dma_start(ck_s[:], cos_k[:])
                        sk_s = nrm.tile([RD, EXT], F32)
                        nc.sync.dma_start(sk_s[:], sinS_k[:])

                        accq = rps.tile([1, 512], F32, tag="row")
                        sqq = nrm.tile([128, TOK], BF16, tag="nsq")
                        for h in range(H):
                            nc.vector.tensor_mul(sqq[:], qT[:, h, :],
                                                 qT[:, h, :])
                            nc.tensor.matmul(accq[:], oneb[:], sqq[:],
                                             start=(h == 0), stop=(h == H - 1))
                        cqb = row_rsqrt_bcast([(accq, 0, 512)], TOK,
                                              H * D, "qn")
                        cqb3 = cqb[:].rearrange("p (g t) -> p g t",
                                                g=1).broadcast_to(
                                                    [128, H, TOK])
                        nc.vector.tensor_mul(qT[:], qT[:], cqb3)
                        for h in range(H):
                            nc.vector.tensor_scalar_mul(
                                qT[:, h, :], qT[:, h, :], qnw_s[:, h:h + 1])

                        acck_lo = rps.tile([1, 512], F32, tag="row")
                        acck_hi = rps.tile([1, 512], F32, tag="row")
                        sqk = nrm.tile([128, EXT], BF16, tag="nsqk")
                        for g in range(HK):
                            nc.vector.tensor_mul(sqk[:], kT[:, g, :],
                                                 kT[:, g, :])
                            nc.tensor.matmul(acck_lo[:], oneb[:],
                                             sqk[:, 0:512],
                                             start=(g == 0), stop=(g == HK - 1))
                            nc.tensor.matmul(acck_hi[:], oneb[:],
                                             sqk[:, 512:1024],
                                             start=(g == 0), stop=(g == HK - 1))
                        ckb = row_rsqrt_bcast(
                            [(acck_lo, 0, 512), (acck_hi, 512, 512)],
                            EXT, HK * D, "kn")
                        ckb3 = ckb[:].rearrange("p (g t) -> p g t",
                                                g=1).broadcast_to(
                                                    [128, HK, EXT])
                        nc.vector.tensor_mul(kT[:], kT[:], ckb3)
                        for g in range(HK):
                            nc.vector.tensor_scalar_mul(
                                kT[:, g, :], kT[:, g, :], knw_s[:, g:g + 1])

                        def rope(t3, nh, width, cos_t, sinS_t):
                            c3 = cos_t[:].rearrange(
                                "p (g t) -> p g t", g=1).broadcast_to(
                                [RD, nh, width])
                            s3 = sinS_t[:].rearrange(
                                "p (g t) -> p g t", g=1).broadcast_to(
                                [RD, nh, width])
                            # partner halves moved onto matching partitions
                            qsw = nrm.tile([RD, nh, width], BF16, tag="rsw")
                            nc.sync.dma_start(qsw[0:32], t3[32:64])
                            nc.sync.dma_start(qsw[32:64], t3[0:32])
                            t1 = nrm.tile([RD, nh, width], BF16, tag="rt1")
                            nc.vector.tensor_mul(t1[:], t3[0:RD], c3)
                            nc.vector.tensor_mul(qsw[:], qsw[:], s3)
                            nc.vector.tensor_add(t3[0:RD], t1[:], qsw[:])

                        rope(qT[:, 0:8, :], 8, TOK, cq_s, sq_s)
                        rope(qT[:, 8:16, :], 8, TOK, cq_s, sq_s)
                        rope(kT[:], HK, EXT, ck_s, sk_s)

                    # ============ phase C: sliding-window attention
                    with tc.tile_pool(name="attn", bufs=1) as ap, \
                         tc.tile_pool(name="es", bufs=16) as es, \
                         tc.tile_pool(name="wop", bufs=2) as wop:
                        # prefetch first o_proj weight quarter during attention
                        NOH = 4
                        OHW = HID // NOH
                        wohs = [None] * NOH

                        def load_woh(oh):
                            t = wop.tile([128, NM, OHW], BF16, tag="wo",
                                         name=f"woh{oh}")
                            nc.sync.dma_start(
                                t[:], woT[:, oh * OHW:(oh + 1) * OHW]
                                .rearrange("(i p) j -> p i j", p=128))
                            wohs[oh] = t

                        load_woh(0)
                        halo_s = ap.tile([128, 8], F32)
                        nc.sync.dma_start(halo_s[:], halo[:])
                        band_s = ap.tile([128, 1408], BF16)
                        nc.sync.dma_start(band_s[:], band[:])
                        attnT = ap.tile([128, H, TOK], BF16)
                        for h in range(H):
                            g = h // (H // HK)
                            exps = []
                            for kt in range(KT):
                                ps = bps.tile([128, 512], F32, tag="big")
                                nc.tensor.matmul(
                                    ps[:], kT[:, g, kt * 128:(kt + 1) * 128],
                                    qT[:, h, :], start=True, stop=True)
                                e = es.tile([128, 512], BF16, tag="e")
                                nc.scalar.activation(
                                    out=e[:], in_=ps[:], func=AF.Exp,
                                    bias=halo_s[:, kt:kt + 1], scale=SCALE)
                                nc.vector.tensor_mul(
                                    e[:], e[:],
                                    band_s[:, 896 - 128 * kt:1408 - 128 * kt])
                                exps.append(e)
                            den = rps.tile([1, 512], F32, tag="row")
                            for kt in range(KT):
                                nc.tensor.matmul(den[:], oneb[:], exps[kt][:],
                                                 start=(kt == 0),
                                                 stop=(kt == KT - 1))
                            drr = sml.tile([1, 512], F32R, tag="drr")
                            with nc.allow_low_precision(reason="f32r ok"):
                                nc.vector.reciprocal(drr[:], den[:])
                            rb = bcast_row(drr, 512, "rden")
                            rbs = sml.tile([128, 512], F32R, tag="rbs")
                            nc.vector.tensor_copy(rbs[:], rb[:])
                            po = bps.tile([128, 512], F32, tag="big")
                            for kt in range(KT):
                                nc.tensor.matmul(
                                    po[:],
                                    Vb[:, kt, g * 128:(g + 1) * 128],
                                    exps[kt][:], start=(kt == 0),
                                    stop=(kt == KT - 1))
                            nc.vector.tensor_mul(attnT[:, h, :], po[:], rbs[:])

                        # ============ phase D: o_proj + residual + ln2
                        with tc.tile_pool(name="x2", bufs=1) as x2p, \
                             tc.tile_pool(name="xs2", bufs=3) as xs2:
                            x2T = x2p.tile([128, NM, TOK], F32)
                            acc2 = rps.tile([1, 512], F32, tag="row")
                            for m in range(NM):
                                oh, mo = divmod(m, NM // NOH)
                                if mo == 0 and oh + 1 < NOH and \
                                        wohs[oh + 1] is None:
                                    load_woh(oh + 1)
                                px = bps.tile([128, 512], F32, tag="big")
                                for i in range(NM):
                                    nc.tensor.matmul(
                                        px[:],
                                        wohs[oh][:, i, mo * 128:(mo + 1) * 128],
                                        attnT[:, i, :],
                                        start=(i == 0), stop=(i == NM - 1))
                                xo = xs2.tile([128, TOK], F32R, tag="xo")
                                nc.scalar.dma_start(
                                    xo[:], xT[m * 128:(m + 1) * 128, 512:1024])
                                nc.vector.tensor_add(x2T[:, m, :], px[:], xo[:])
                                sq2 = xs2.tile([128, TOK], F32R, tag="sq2")
                                nc.vector.tensor_mul(sq2[:], x2T[:, m, :],
                                                     x2T[:, m, :])
                                nc.tensor.matmul(acc2[:], onec[:], sq2[:],
                                                 start=(m == 0),
                                                 stop=(m == NM - 1))

                            # x2 token-major -> DRAM (for final residual)
                            for tt in range(4):
                                for grp in range(4):
                                    ts = xs2.tile([128, 512], F32, tag="x2t")
                                    for j in range(4):
                                        m = grp * 4 + j
                                        pt = bps.tile([128, 128], F32,
                                                      tag="big")
                                        nc.tensor.transpose(
                                            pt[:],
                                            x2T[:, m, tt * 128:(tt + 1) * 128],
                                            identF[:])
                                        nc.vector.tensor_copy(
                                            ts[:, j * 128:(j + 1) * 128],
                                            pt[:])
                                    nc.scalar.dma_start(
                                        x2tok[tt * 128:(tt + 1) * 128,
                                              grp * 512:(grp + 1) * 512],
                                        ts[:])

                            s2b = row_rsqrt_bcast([(acc2, 0, 512)], TOK,
                                                  HID, "l2")
                            for m in range(NM):
                                h2t = xs2.tile([128, TOK], BF16, tag="h2t")
                                nc.vector.tensor_mul(h2t[:], x2T[:, m, :],
                                                     s2b[:])
                                nc.vector.tensor_scalar_mul(
                                    h2s[:, m, :], h2t[:], ln2w_s[:, m:m + 1])

            # ============ phase E: data-parallel MLP, streamed bf16 weights
            with tc.tile_pool(name="gpool", bufs=1) as gpo:
                gt = gpo.tile([128, NMI, TOK], BF16)

                with tc.tile_pool(name="slab", bufs=2) as slp, \
                     tc.tile_pool(name="silp", bufs=3) as silp, \
                     tc.tile_pool(name="eps", bufs=4, space="PSUM") as eps:
                    for c in range(NCH1):
                        w1s = slp.tile([128, NM, CW], BF16, tag="w1s")
                        w3s = slp.tile([128, NM, CW], BF16, tag="w3s")
                        nc.sync.dma_start(
                            w1s[:], w1P[c].rearrange("i p j -> p i j"))
                        nc.sync.dma_start(
                            w3s[:], w3P[c].rearrange("i p j -> p i j"))
                        for mm in range(CW // 128):
                            mi = c * (CW // 128) + mm
                            pa = eps.tile([128, 512], F32, tag="e")
                            for i in range(NM):
                                nc.tensor.matmul(
                                    pa[:], w1s[:, i, mm * 128:(mm + 1) * 128],
                                    h2s[:, i, :],
                                    start=(i == 0), stop=(i == NM - 1))
                            pb = eps.tile([128, 512], F32, tag="e")
                            for i in range(NM):
                                nc.tensor.matmul(
                                    pb[:], w3s[:, i, mm * 128:(mm + 1) * 128],
                                    h2s[:, i, :],
                                    start=(i == 0), stop=(i == NM - 1))
                            sil = silp.tile([128, 512], F32, tag="sil")
                            nc.scalar.activation(out=sil[:], in_=pa[:],
                                                 func=AF.Silu)
                            nc.vector.tensor_mul(gt[:, mi, :], sil[:], pb[:])

                # pass 2: out = g @ w2 + x2   (w2 streamed once)
                MH = NMI // 2   # 32 im strips per w2 slab
                with tc.tile_pool(name="w2s", bufs=2) as w2sp, \
                     tc.tile_pool(name="ops", bufs=4, space="PSUM") as ops, \
                     tc.tile_pool(name="top", bufs=4) as top:
                    for hs in range(HS):
                        outp = [ops.tile([128, HSW], F32, tag="o",
                                         name=f"outp{hs}_{tt}")
                                for tt in range(4)]
                        x2ss = []
                        for tt in range(4):
                            x2s = top.tile([128, HSW], F32, tag="x2s",
                                           name=f"x2s{hs}_{tt}")
                            nc.sync.dma_start(
                                x2s[:], x2tok[tt * 128:(tt + 1) * 128,
                                              hs * HSW:(hs + 1) * HSW])
                            x2ss.append(x2s)
                        for mh in range(2):
                            w2t = w2sp.tile([128, MH, HSW], BF16, tag="w2t")
                            nc.sync.dma_start(
                                w2t[:], w2P[hs, mh * MH:(mh + 1) * MH]
                                .rearrange("m p j -> p m j"))
                            for mm in range(MH):
                                m = mh * MH + mm
                                for tt in range(4):
                                    nc.tensor.matmul(
                                        outp[tt][:],
                                        gt[:, m, tt * 128:(tt + 1) * 128],
                                        w2t[:, mm, :],
                                        start=(m == 0), stop=(m == NMI - 1))
                        for tt in range(4):
                            os_ = top.tile([128, HSW], F32, tag="os")
                            nc.vector.tensor_add(os_[:], outp[tt][:],
                                                 x2ss[tt][:])
                            nc.sync.dma_start(
                                out[tt * 128:(tt + 1) * 128,
                                    hs * HSW:(hs + 1) * HSW], os_[:])

    return nc


# ---------------------------------------------------------------- host side
def _rope_tables(pos):
    inv = 1.0 / (THETA ** (np.arange(0, RD, 2, dtype=np.float32) / RD))
    f = pos[:, None].astype(np.float32) * inv[None, :]
    emb = np.concatenate([f, f], axis=-1)          # [T, RD]
    cos = np.ascontiguousarray(np.cos(emb).T)      # [RD, T]
    sin = np.sin(emb).T
    sinS = sin.copy()
    sinS[0:32] = -sin[0:32]
    return cos.astype(np.float32), np.ascontiguousarray(sinS).astype(np.float32)


def _band_mask():
    import ml_dtypes
    p = np.arange(128)[:, None]
    u = np.arange(1408)[None, :]
    m = ((u >= p + 384) & (u <= p + 896)).astype(np.float32)
    return m.astype(ml_dtypes.bfloat16)


def _prepare_in_maps(hidden_states, wq, wk, wv, wo, q_norm_w, k_norm_w,
                     ln1_w, ln2_w, w1, w2, w3):
    import ml_dtypes
    bf = ml_dtypes.bfloat16
    xf = np.ascontiguousarray(hidden_states.reshape(B * S, HID))
    wqTn = np.ascontiguousarray(wq.T.astype(bf))
    wkTn = np.ascontiguousarray(wk.T.astype(bf))
    wvTn = np.ascontiguousarray(wv.T.astype(bf))
    woTn = np.ascontiguousarray(wo.T.astype(bf))
    w1Pn = np.ascontiguousarray(
        w1.T.reshape(NM, 128, NCH1, CW).transpose(2, 0, 1, 3).astype(bf))
    w3Pn = np.ascontiguousarray(
        w3.T.reshape(NM, 128, NCH1, CW).transpose(2, 0, 1, 3).astype(bf))
    w2Pn = np.ascontiguousarray(
        w2.T.reshape(NMI, 128, HS, HSW).transpose(2, 0, 1, 3).astype(bf))
    ln1c = np.ascontiguousarray(ln1_w.reshape(16, 128).T)
    ln2c = np.ascontiguousarray(ln2_w.reshape(16, 128).T)
    qnc = np.ascontiguousarray(q_norm_w.reshape(16, 128).T)
    knc = np.ascontiguousarray(k_norm_w.reshape(4, 128).T)
    band = _band_mask()
    ones_c = np.ones((128, 1), np.float32)
    ones_r = np.ones((1, 128), np.float32)
    ones_b = np.ones((128, 1), ml_dtypes.bfloat16)

    in_maps = []
    for c in range(NCORES):
        t0 = c * TOK
        bidx = t0 // S
        s0 = t0 % S
        xe = np.zeros((EXT, HID), np.float32)
        lo = s0 - WIN
        if lo >= 0:
            xe[:] = xf[bidx * S + lo: bidx * S + s0 + TOK]
            halo_valid = True
        else:
            xe[WIN:] = xf[bidx * S + s0: bidx * S + s0 + TOK]
            halo_valid = False
        xTc = np.ascontiguousarray(xe.T)

        qpos = np.arange(s0, s0 + TOK)
        kpos = np.arange(s0 - WIN, s0 + TOK)
        cq, sq_ = _rope_tables(qpos)
        ck, sk_ = _rope_tables(np.maximum(kpos, 0))
        halo_bias = np.zeros(EXT, np.float32)
        if not halo_valid:
            halo_bias[0:WIN] = NEG
        haloc = np.ascontiguousarray(halo_bias.reshape(8, 128).T)

        in_maps.append({
            "xT": xTc,
            "wqT": wqTn, "wkT": wkTn, "wvT": wvTn, "woT": woTn,
            "w1P": w1Pn, "w3P": w3Pn, "w2P": w2Pn,
            "ln1w": ln1c, "ln2w": ln2c, "qnw": qnc, "knw": knc,
            "cos_q": cq, "sinS_q": sq_, "cos_k": ck, "sinS_k": sk_,
            "halo": haloc, "band": band,
            "ones_c": ones_c, "ones_r": ones_r, "ones_b": ones_b,
        })
    return in_maps


_NC = None


def _get_nc():
    global _NC
    if _NC is None:
        _register_ntff_hook()
        _NC = build_nc()
    return _NC


def run(in_maps, trace=False):
    from concourse.bass_utils import run_bass_kernel_spmd
    nc = _get_nc()
    return run_bass_kernel_spmd(nc, in_maps, core_ids=list(range(NCORES)),
                                trace=trace)


def kernel(**inputs):
    arrs = {k: np.asarray(v, dtype=np.float32) for k, v in inputs.items()}
    in_maps = _prepare_in_maps(
        arrs["hidden_states"], arrs["wq"], arrs["wk"], arrs["wv"], arrs["wo"],
        arrs["q_norm_w"], arrs["k_norm_w"], arrs["ln1_w"], arrs["ln2_w"],
        arrs["w1"], arrs["w2"], arrs["w3"])
    res = run(in_maps, trace=False)
    full = np.empty((B * S, HID), np.float32)
    for c in range(NCORES):
        full[c * TOK:(c + 1) * TOK] = res.results[c]["out"]
    return full.reshape(B, S, HID)


# revision 33
# speedup vs baseline: 1.1857x; 1.1857x over previous
"""MiniMax-M2 decoder layer on 8 trn2 NeuronCores (v2).

Sharding: fully sequence-sharded. Each core owns 512 tokens of the
flattened (B*S)=4096 token stream and recomputes the 512-token KV halo
locally (no collectives in the attention block). The MLP is
data-parallel over tokens too: every core streams the full bf16
w1/w3/w2 from its DRAM once (~100MB, hidden under the 655us of MLP
matmul) while computing only its own 512 tokens -> ZERO collectives in
the whole kernel.

All weights are pre-cast to bf16 and pre-packed on the host so every
weight DMA is a large contiguous block (128KB-512KB, 1-4KB per
partition line).

Matmuls run in bf16 against f32 psum. Softmax skips the
max-subtraction (scores are bounded ~6 for this layer's magnitudes).

Self-contained: includes the BIR wait-splitting fix this container's
walrus build needs (1 semaphore wait per instruction max).
"""

import json
import sys
import types

import numpy as np

import concourse.bass as bass
import concourse.mybir as mybir
import concourse.tile as tile
from concourse.masks import make_identity

# ---------------------------------------------------------------- constants
B, S, HID = 2, 2048, 2048
H, HK, D = 16, 4, 128
RD = 64
IM = 8192
WIN = 512
EPS = 1e-6
THETA = 10000.0
SCALE = D ** -0.5

NCORES = 8
TOK = 512              # own tokens per core
EXT = 1024             # halo + own
NEG = -1e9

F32 = mybir.dt.float32
F32R = mybir.dt.float32r
BF16 = mybir.dt.bfloat16
AF = mybir.ActivationFunctionType

KT = 8                 # 128-wide key tiles over EXT
NM = HID // 128        # 16 hid tiles
CW = 512               # im chunk width for w1/w3 streaming
NCH1 = IM // CW        # 16 chunks
NMI = IM // 128        # 64 im tiles
HS = 4                 # hid column quarters for w2 streaming
HSW = HID // HS        # 512

# ------------------------------------------------------- walrus wait-split fix
MAX_WAITS = 1


def _split_excess_waits(bir_bytes: bytes) -> bytes:
    m = json.loads(bir_bytes)
    ctr = [0]

    def fix_insts(insts):
        out = []
        for ins in insts:
            si = ins.get("sync_info")
            ow = (si or {}).get("on_wait") or []
            if len(ow) > MAX_WAITS:
                eng = ins["engine"]
                keep = ow[-MAX_WAITS:]
                excess = ow[:-MAX_WAITS]
                ins["sync_info"]["on_wait"] = keep
                for i in range(0, len(excess), MAX_WAITS):
                    ctr[0] += 1
                    out.append({
                        "debug": ins.get("debug", 0),
                        "engine": eng,
                        "ins": [],
                        "name": f"I-waitfix-{ctr[0]}",
                        "opcode": "NoOp",
                        "outs": [],
                        "sync_info": {"on_update": [],
                                      "on_wait": excess[i:i + MAX_WAITS]},
                        "text_hint": "waitfix",
                    })
            out.append(ins)
        return out

    def walk(o):
        if isinstance(o, dict):
            if isinstance(o.get("instructions"), list):
                o["instructions"] = fix_insts(o["instructions"])
            for v in o.values():
                walk(v)
        elif isinstance(o, list):
            for v in o:
                walk(v)

    walk(m)
    return json.dumps(m).encode()


class _BassFixed(bass.Bass):
    def to_json_bytes(self) -> bytes:
        return _split_excess_waits(super().to_json_bytes())


def _register_ntff_hook():
    """Provide antenv.axon_hooks (missing in this image) so trace=True works."""
    if "antenv.axon_hooks" in sys.modules:
        return
    try:
        import trn_agent_boot.trn_boot as tb
    except ImportError:
        return
    mod = types.ModuleType("antenv.axon_hooks")
    holder = [None]
    mod.set_axon_ntff_profile_hook = lambda h: holder.__setitem__(0, h)
    mod.get_axon_ntff_profile_hook = lambda: holder[0]
    sys.modules["antenv.axon_hooks"] = mod
    try:
        mod.set_axon_ntff_profile_hook(
            tb._ntff_profile_via_ctypes("/opt/axon/libaxon_pjrt.so"))
    except Exception:
        pass


# ---------------------------------------------------------------- the program
def build_nc():
    nc = _BassFixed(num_devices=NCORES, target_bir_lowering=False)

    xT = nc.dram_tensor("xT", [HID, EXT], F32R, kind="ExternalInput")
    wqT = nc.dram_tensor("wqT", [HID, H * D], BF16, kind="ExternalInput")
    wkT = nc.dram_tensor("wkT", [HID, HK * D], BF16, kind="ExternalInput")
    wvT = nc.dram_tensor("wvT", [HID, HK * D], BF16, kind="ExternalInput")
    woT = nc.dram_tensor("woT", [H * D, HID], BF16, kind="ExternalInput")
    w1P = nc.dram_tensor("w1P", [NCH1, NM, 128, CW], BF16,
                         kind="ExternalInput")
    w3P = nc.dram_tensor("w3P", [NCH1, NM, 128, CW], BF16,
                         kind="ExternalInput")
    w2P = nc.dram_tensor("w2P", [HS, NMI, 128, HSW], BF16,
                         kind="ExternalInput")
    ln1w = nc.dram_tensor("ln1w", [128, 16], F32, kind="ExternalInput")
    ln2w = nc.dram_tensor("ln2w", [128, 16], F32, kind="ExternalInput")
    qnw = nc.dram_tensor("qnw", [128, 16], F32, kind="ExternalInput")
    knw = nc.dram_tensor("knw", [128, 4], F32, kind="ExternalInput")
    cos_q = nc.dram_tensor("cos_q", [RD, TOK], F32, kind="ExternalInput")
    sinS_q = nc.dram_tensor("sinS_q", [RD, TOK], F32, kind="ExternalInput")
    cos_k = nc.dram_tensor("cos_k", [RD, EXT], F32, kind="ExternalInput")
    sinS_k = nc.dram_tensor("sinS_k", [RD, EXT], F32, kind="ExternalInput")
    halo = nc.dram_tensor("halo", [128, 8], F32, kind="ExternalInput")
    bandm = nc.dram_tensor("bandm", [128, 4, 256], BF16, kind="ExternalInput")
    ones_c = nc.dram_tensor("ones_c", [128, 1], F32R, kind="ExternalInput")
    ones_r = nc.dram_tensor("ones_r", [1, 128], F32R, kind="ExternalInput")
    ones_b = nc.dram_tensor("ones_b", [128, 1], BF16, kind="ExternalInput")

    out = nc.dram_tensor("out", [TOK, HID], F32, kind="ExternalOutput")

    with tile.TileContext(nc) as tc:
        with tc.tile_pool(name="consts", bufs=1) as cst, \
             tc.tile_pool(name="smalls", bufs=2) as sml, \
             tc.tile_pool(name="h2keep", bufs=1) as h2keep, \
             tc.tile_pool(name="dram", bufs=1, space="DRAM") as dram:

            # ---------------- constants
            onec = cst.tile([128, 1], F32R)
            nc.sync.dma_start(onec[:], ones_c[:])
            oner = cst.tile([1, 128], F32R)
            nc.sync.dma_start(oner[:], ones_r[:])
            oneb = cst.tile([128, 1], BF16)
            nc.sync.dma_start(oneb[:], ones_b[:])
            ln1w_s = cst.tile([128, 16], F32)
            nc.sync.dma_start(ln1w_s[:], ln1w[:])
            ln2w_s = cst.tile([128, 16], F32)
            nc.sync.dma_start(ln2w_s[:], ln2w[:])
            qnw_s = cst.tile([128, 16], F32)
            nc.sync.dma_start(qnw_s[:], qnw[:])
            knw_s = cst.tile([128, 4], F32)
            nc.sync.dma_start(knw_s[:], knw[:])
            eps_s = cst.tile([1, 1], F32)
            nc.vector.memset(eps_s[:], EPS)
            identF = cst.tile([128, 128], F32)
            make_identity(nc, identF[:])
            identR = cst.tile([128, 128], F32R)
            nc.vector.tensor_copy(identR[:], identF[:])

            h2s = h2keep.tile([128, NM, TOK], BF16)
            x2tok = dram.tile([TOK, HID], F32)

            with tc.tile_pool(name="rowps", bufs=3, space="PSUM") as rps, \
                 tc.tile_pool(name="bigps", bufs=3, space="PSUM") as bps, \
                 tc.tile_pool(name="bcps", bufs=1, space="PSUM") as bcps:

                def bcast_row(row_f32r, width, tag):
                    """[1,width] F32R row -> [128,width] F32 psum broadcast."""
                    p = bcps.tile([128, width], F32, tag="bc")
                    for j in range(0, width, 512):
                        w = min(512, width - j)
                        nc.tensor.matmul(p[:, j:j + w], oner[:],
                                         row_f32r[:, j:j + w],
                                         start=True, stop=True)
                    return p

                def row_rsqrt_bcast(acc_parts, width, denom, tag):
                    """psum [1,*] sum-of-squares parts -> [128,width] psum with
                    1/sqrt(acc/denom + eps) broadcast over partitions."""
                    srow = sml.tile([1, width], F32, tag="srow")
                    for (p, j, w) in acc_parts:
                        nc.scalar.activation(out=srow[:, j:j + w], in_=p[:],
                                             func=AF.Sqrt, bias=eps_s[:],
                                             scale=1.0 / denom)
                    rrow = sml.tile([1, width], F32R, tag="rrow")
                    with nc.allow_low_precision(reason="f32r rounding ok"):
                        nc.vector.reciprocal(rrow[:], srow[:])
                    return bcast_row(rrow, width, tag)

                # ======== q/k/v activations
                with tc.tile_pool(name="qkv", bufs=1) as qkv:
                    qT = qkv.tile([128, H, TOK], BF16)
                    kT = qkv.tile([128, HK, EXT], BF16)
                    Vb = qkv.tile([128, KT, HK * D], BF16)

                    # ============ phase A+B: ln1-RMS + Q/K/V projections
                    with tc.tile_pool(name="hp", bufs=1) as hp, \
                         tc.tile_pool(name="xs", bufs=4) as xs, \
                         tc.tile_pool(name="sqp", bufs=2) as sqp, \
                         tc.tile_pool(name="wat", bufs=1) as wat:
                        vT = hp.tile([128, HK, EXT], F32R, tag="vT")
                        hTh = hp.tile([128, NM, EXT], BF16, tag="h")

                        # resident attention weights (big contiguous strips)
                        wks = wat.tile([128, NM, HK * D], BF16)
                        wvs = wat.tile([128, NM, HK * D], BF16)
                        nc.sync.dma_start(
                            wks[:], wkT[:].rearrange("(i p) j -> p i j", p=128))
                        nc.sync.dma_start(
                            wvs[:], wvT[:].rearrange("(i p) j -> p i j", p=128))

                        for half in (0, 1):
                            c0 = half * 512
                            acc = rps.tile([1, 512], F32, tag="row")
                            for i in range(NM):
                                xt = xs.tile([128, 512], F32R, tag="xt")
                                nc.scalar.dma_start(
                                    xt[:],
                                    xT[i * 128:(i + 1) * 128, c0:c0 + 512])
                                sq = sqp.tile([128, 512], F32R, tag="sq")
                                nc.vector.tensor_mul(sq[:], xt[:], xt[:])
                                nc.tensor.matmul(acc[:], onec[:], sq[:],
                                                 start=(i == 0),
                                                 stop=(i == NM - 1))
                            s1b = row_rsqrt_bcast([(acc, 0, 512)], 512,
                                                  HID, "l1")
                            for i in range(NM):
                                xt = xs.tile([128, 512], F32R, tag="xt")
                                nc.scalar.dma_start(
                                    xt[:],
                                    xT[i * 128:(i + 1) * 128, c0:c0 + 512])
                                tmp = sqp.tile([128, 512], F32R, tag="sq")
                                nc.vector.tensor_mul(tmp[:], xt[:], s1b[:])
                                nc.vector.tensor_scalar_mul(
                                    hTh[:, i, c0:c0 + 512], tmp[:],
                                    ln1w_s[:, i:i + 1])

                            # K / V for this half
                            for m in range(HK):
                                pk = bps.tile([128, 512], F32, tag="big")
                                for i in range(NM):
                                    nc.tensor.matmul(
                                        pk[:], wks[:, i, m * 128:(m + 1) * 128],
                                        hTh[:, i, c0:c0 + 512],
                                        start=(i == 0), stop=(i == NM - 1))
                                pv = bps.tile([128, 512], F32, tag="big")
                                for i in range(NM):
                                    nc.tensor.matmul(
                                        pv[:], wvs[:, i, m * 128:(m + 1) * 128],
                                        hTh[:, i, c0:c0 + 512],
                                        start=(i == 0), stop=(i == NM - 1))
                                nc.vector.tensor_copy(kT[:, m, c0:c0 + 512],
                                                      pk[:])
                                nc.vector.tensor_copy(vT[:, m, c0:c0 + 512],
                                                      pv[:])

                        # Q projection (own tokens only), wq in two halves
                        for qh in range(2):
                            wqs = wat.tile([128, NM, H * D // 2], BF16,
                                           tag="wq", bufs=1,
                                           name=f"wqs{qh}")
                            nc.sync.dma_start(
                                wqs[:],
                                wqT[:, qh * 1024:(qh + 1) * 1024]
                                .rearrange("(i p) j -> p i j", p=128))
                            for mm in range(H // 2):
                                m = qh * 8 + mm
                                pq = bps.tile([128, 512], F32, tag="big")
                                for i in range(NM):
                                    nc.tensor.matmul(
                                        pq[:],
                                        wqs[:, i, mm * 128:(mm + 1) * 128],
                                        hTh[:, i, 512:1024],
                                        start=(i == 0), stop=(i == NM - 1))
                                nc.vector.tensor_copy(qT[:, m, :], pq[:])

                        # transpose vT -> token-major bf16 Vb
                        for kt in range(KT):
                            for g in range(HK):
                                pt = bps.tile([128, 128], F32R, tag="big")
                                nc.tensor.transpose(
                                    pt[:], vT[:, g, kt * 128:(kt + 1) * 128],
                                    identR[:])
                                nc.vector.tensor_copy(
                                    Vb[:, kt, g * 128:(g + 1) * 128], pt[:])

                    # ---------------- fused q/k RMSNorm + partial RoPE
                    with tc.tile_pool(name="nrm", bufs=1) as nrm:
                        cq_s = nrm.tile([RD, TOK], F32)
                        nc.sync.dma_start(cq_s[:], cos_q[:])
                        sq_s = nrm.tile([RD, TOK], F32)
                        nc.sync.dma_start(sq_s[:], sinS_q[:])
                        ck_s = nrm.tile([RD, EXT], F32)
                        nc.sync.dma_start(ck_s[:], cos_k[:])
                        sk_s = nrm.tile([RD, EXT], F32)
                        nc.sync.dma_start(sk_s[:], sinS_k[:])

                        accq = rps.tile([1, 512], F32, tag="row")
                        sqq = nrm.tile([128, TOK], BF16, tag="nsq")
                        for h in range(H):
                            nc.vector.tensor_mul(sqq[:], qT[:, h, :],
                                                 qT[:, h, :])
                            nc.tensor.matmul(accq[:], oneb[:], sqq[:],
                                             start=(h == 0), stop=(h == H - 1))
                        cqb = row_rsqrt_bcast([(accq, 0, 512)], TOK,
                                              H * D, "qn")
                        cqb3 = cqb[:].rearrange("p (g t) -> p g t",
                                                g=1).broadcast_to(
                                                    [128, H, TOK])
                        nc.vector.tensor_mul(qT[:], qT[:], cqb3)
                        for h in range(H):
                            nc.vector.tensor_scalar_mul(
                                qT[:, h, :], qT[:, h, :], qnw_s[:, h:h + 1])

                        acck_lo = rps.tile([1, 512], F32, tag="row")
                        acck_hi = rps.tile([1, 512], F32, tag="row")
                        sqk = nrm.tile([128, EXT], BF16, tag="nsqk")
                        for g in range(HK):
                            nc.vector.tensor_mul(sqk[:], kT[:, g, :],
                                                 kT[:, g, :])
                            nc.tensor.matmul(acck_lo[:], oneb[:],
                                             sqk[:, 0:512],
                                             start=(g == 0), stop=(g == HK - 1))
                            nc.tensor.matmul(acck_hi[:], oneb[:],
                                             sqk[:, 512:1024],
                                             start=(g == 0), stop=(g == HK - 1))
                        ckb = row_rsqrt_bcast(
                            [(acck_lo, 0, 512), (acck_hi, 512, 512)],
                            EXT, HK * D, "kn")
                        ckb3 = ckb[:].rearrange("p (g t) -> p g t",
                                                g=1).broadcast_to(
                                                    [128, HK, EXT])
                        nc.vector.tensor_mul(kT[:], kT[:], ckb3)
                        for g in range(HK):
                            nc.vector.tensor_scalar_mul(
                                kT[:, g, :], kT[:, g, :], knw_s[:, g:g + 1])

                        def rope(t3, nh, width, cos_t, sinS_t):
                            c3 = cos_t[:].rearrange(
                                "p (g t) -> p g t", g=1).broadcast_to(
                                [RD, nh, width])
                            s3 = sinS_t[:].rearrange(
                                "p (g t) -> p g t", g=1).broadcast_to(
                                [RD, nh, width])
                            # partner halves moved onto matching partitions
                            qsw = nrm.tile([RD, nh, width], BF16, tag="rsw")
                            nc.sync.dma_start(qsw[0:32], t3[32:64])
                            nc.sync.dma_start(qsw[32:64], t3[0:32])
                            t1 = nrm.tile([RD, nh, width], BF16, tag="rt1")
                            nc.vector.tensor_mul(t1[:], t3[0:RD], c3)
                            nc.vector.tensor_mul(qsw[:], qsw[:], s3)
                            nc.vector.tensor_add(t3[0:RD], t1[:], qsw[:])

                        rope(qT[:, 0:8, :], 8, TOK, cq_s, sq_s)
                        rope(qT[:, 8:16, :], 8, TOK, cq_s, sq_s)
                        rope(kT[:], HK, EXT, ck_s, sk_s)

                    # ============ phase C: sliding-window attention
                    with tc.tile_pool(name="attn", bufs=1) as ap, \
                         tc.tile_pool(name="es", bufs=16) as es, \
                         tc.tile_pool(name="wop", bufs=2) as wop:
                        # prefetch first o_proj weight quarter during attention
                        NOH = 4
                        OHW = HID // NOH
                        wohs = [None] * NOH

                        def load_woh(oh):
                            t = wop.tile([128, NM, OHW], BF16, tag="wo",
                                         name=f"woh{oh}")
                            nc.sync.dma_start(
                                t[:], woT[:, oh * OHW:(oh + 1) * OHW]
                                .rearrange("(i p) j -> p i j", p=128))
                            wohs[oh] = t

                        load_woh(0)
                        halo_s = ap.tile([128, 8], F32)
                        nc.sync.dma_start(halo_s[:], halo[:])
                        bandm_s = ap.tile([128, 4, 256], BF16)
                        nc.sync.dma_start(bandm_s[:], bandm[:])
                        attnT = ap.tile([128, H, TOK], BF16)
                        # banded: query chunks of 256 see only 6 key tiles
                        MIDX = {0: 0, 1: 1, 4: 2, 5: 3}
                        for h in range(H):
                            g = h // (H // HK)
                            for j2 in range(2):
                                q0 = j2 * 256
                                es_l = []
                                for u in range(6):
                                    kt = 2 * j2 + u
                                    ps = bps.tile([128, 256], F32, tag="big",
                                                  name=f"ps{h}_{j2}_{u}")
                                    nc.tensor.matmul(
                                        ps[:],
                                        kT[:, g, kt * 128:(kt + 1) * 128],
                                        qT[:, h, q0:q0 + 256],
                                        start=True, stop=True)
                                    e = es.tile([128, 256], BF16, tag="e",
                                                name=f"e{h}_{j2}_{u}")
                                    nc.scalar.activation(
                                        out=e[:], in_=ps[:], func=AF.Exp,
                                        bias=halo_s[:, kt:kt + 1], scale=SCALE)
                                    if u in MIDX:
                                        nc.vector.tensor_mul(
                                            e[:], e[:],
                                            bandm_s[:, MIDX[u], :])
                                    es_l.append(e)
                                # elementwise sum over the 6 key tiles, then
                                # one matmul reduces over the 128 partitions
                                t1 = es.tile([128, 256], BF16, tag="t",
                                             name=f"t1_{h}_{j2}")
                                nc.vector.tensor_add(t1[:], es_l[0][:],
                                                     es_l[1][:])
                                t2 = es.tile([128, 256], BF16, tag="t",
                                             name=f"t2_{h}_{j2}")
                                nc.vector.tensor_add(t2[:], es_l[2][:],
                                                     es_l[3][:])
                                t3 = es.tile([128, 256], BF16, tag="t",
                                             name=f"t3_{h}_{j2}")
                                nc.vector.tensor_add(t3[:], es_l[4][:],
                                                     es_l[5][:])
                                t4 = es.tile([128, 256], BF16, tag="t",
                                             name=f"t4_{h}_{j2}")
                                nc.vector.tensor_add(t4[:], t1[:], t2[:])
                                t5 = es.tile([128, 256], BF16, tag="t",
                                             name=f"t5_{h}_{j2}")
                                nc.vector.tensor_add(t5[:], t4[:], t3[:])
                                den = rps.tile([1, 256], F32, tag="row",
                                               name=f"den{h}_{j2}")
                                nc.tensor.matmul(den[:], oneb[:], t5[:],
                                                 start=True, stop=True)
                                po = bps.tile([128, 256], F32, tag="big",
                                              name=f"po{h}_{j2}")
                                for idx, e in enumerate(es_l):
                                    kt = 2 * j2 + idx
                                    nc.tensor.matmul(
                                        po[:],
                                        Vb[:, kt, g * 128:(g + 1) * 128],
                                        e[:], start=(idx == 0),
                                        stop=(idx == 5))
                                drr = sml.tile([1, 256], F32R, tag="drr",
                                               name=f"drr{h}_{j2}")
                                with nc.allow_low_precision(reason="f32r ok"):
                                    nc.vector.reciprocal(drr[:], den[:])
                                rb = bcast_row(drr, 256, "rden")
                                rbs = sml.tile([128, 256], F32R, tag="rbs",
                                               name=f"rbs{h}_{j2}")
                                nc.vector.tensor_copy(rbs[:], rb[:])
                                nc.vector.tensor_mul(
                                    attnT[:, h, q0:q0 + 256], po[:], rbs[:])

                        # ============ phase D: o_proj + residual + ln2
                        with tc.tile_pool(name="x2", bufs=1) as x2p, \
                             tc.tile_pool(name="xs2", bufs=3) as xs2:
                            x2T = x2p.tile([128, NM, TOK], F32)
                            acc2 = rps.tile([1, 512], F32, tag="row")
                            for m in range(NM):
                                oh, mo = divmod(m, NM // NOH)
                                if mo == 0 and oh + 1 < NOH and \
                                        wohs[oh + 1] is None:
                                    load_woh(oh + 1)
                                px = bps.tile([128, 512], F32, tag="big")
                                for i in range(NM):
                                    nc.tensor.matmul(
                                        px[:],
                                        wohs[oh][:, i, mo * 128:(mo + 1) * 128],
                                        attnT[:, i, :],
                                        start=(i == 0), stop=(i == NM - 1))
                                xo = xs2.tile([128, TOK], F32R, tag="xo")
                                nc.scalar.dma_start(
                                    xo[:], xT[m * 128:(m + 1) * 128, 512:1024])
                                nc.vector.tensor_add(x2T[:, m, :], px[:], xo[:])
                                sq2 = xs2.tile([128, TOK], F32R, tag="sq2")
                                nc.vector.tensor_mul(sq2[:], x2T[:, m, :],
                                                     x2T[:, m, :])
                                nc.tensor.matmul(acc2[:], onec[:], sq2[:],
                                                 start=(m == 0),
                                                 stop=(m == NM - 1))

                            # x2 token-major -> DRAM (for final residual)
                            for tt in range(4):
                                for grp in range(4):
                                    ts = xs2.tile([128, 512], F32, tag="x2t")
                                    for j in range(4):
                                        m = grp * 4 + j
                                        pt = bps.tile([128, 128], F32,
                                                      tag="big")
                                        nc.tensor.transpose(
                                            pt[:],
                                            x2T[:, m, tt * 128:(tt + 1) * 128],
                                            identF[:])
                                        nc.vector.tensor_copy(
                                            ts[:, j * 128:(j + 1) * 128],
                                            pt[:])
                                    nc.scalar.dma_start(
                                        x2tok[tt * 128:(tt + 1) * 128,
                                              grp * 512:(grp + 1) * 512],
                                        ts[:])

                            s2b = row_rsqrt_bcast([(acc2, 0, 512)], TOK,
                                                  HID, "l2")
                            for m in range(NM):
                                h2t = xs2.tile([128, TOK], BF16, tag="h2t")
                                nc.vector.tensor_mul(h2t[:], x2T[:, m, :],
                                                     s2b[:])
                                nc.vector.tensor_scalar_mul(
                                    h2s[:, m, :], h2t[:], ln2w_s[:, m:m + 1])

            # ============ phase E: data-parallel MLP, streamed bf16 weights
            with tc.tile_pool(name="gpool", bufs=1) as gpo:
                gt = gpo.tile([128, NMI, TOK], BF16)

                with tc.tile_pool(name="slab", bufs=2) as slp, \
                     tc.tile_pool(name="silp", bufs=3) as silp, \
                     tc.tile_pool(name="eps", bufs=4, space="PSUM") as eps:
                    for c in range(NCH1):
                        w1s = slp.tile([128, NM, CW], BF16, tag="w1s")
                        w3s = slp.tile([128, NM, CW], BF16, tag="w3s")
                        nc.sync.dma_start(
                            w1s[:], w1P[c].rearrange("i p j -> p i j"))
                        nc.sync.dma_start(
                            w3s[:], w3P[c].rearrange("i p j -> p i j"))
                        for mm in range(CW // 128):
                            mi = c * (CW // 128) + mm
                            pa = eps.tile([128, 512], F32, tag="e")
                            for i in range(NM):
                                nc.tensor.matmul(
                                    pa[:], w1s[:, i, mm * 128:(mm + 1) * 128],
                                    h2s[:, i, :],
                                    start=(i == 0), stop=(i == NM - 1))
                            pb = eps.tile([128, 512], F32, tag="e")
                            for i in range(NM):
                                nc.tensor.matmul(
                                    pb[:], w3s[:, i, mm * 128:(mm + 1) * 128],
                                    h2s[:, i, :],
                                    start=(i == 0), stop=(i == NM - 1))
                            sil = silp.tile([128, 512], F32, tag="sil")
                            nc.scalar.activation(out=sil[:], in_=pa[:],
                                                 func=AF.Silu)
                            nc.vector.tensor_mul(gt[:, mi, :], sil[:], pb[:])

                # pass 2: out = g @ w2 + x2   (w2 streamed once)
                MH = NMI // 2   # 32 im strips per w2 slab
                with tc.tile_pool(name="w2s", bufs=2) as w2sp, \
                     tc.tile_pool(name="ops", bufs=4, space="PSUM") as ops, \
                     tc.tile_pool(name="top", bufs=4) as top:
                    for hs in range(HS):
                        outp = [ops.tile([128, HSW], F32, tag="o",
                                         name=f"outp{hs}_{tt}")
                                for tt in range(4)]
                        x2ss = []
                        for tt in range(4):
                            x2s = top.tile([128, HSW], F32, tag="x2s",
                                           name=f"x2s{hs}_{tt}")
                            nc.sync.dma_start(
                                x2s[:], x2tok[tt * 128:(tt + 1) * 128,
                                              hs * HSW:(hs + 1) * HSW])
                            x2ss.append(x2s)
                        for mh in range(2):
                            w2t = w2sp.tile([128, MH, HSW], BF16, tag="w2t")
                            nc.sync.dma_start(
                                w2t[:], w2P[hs, mh * MH:(mh + 1) * MH]
                                .rearrange("m p j -> p m j"))
                            for mm in range(MH):
                                m = mh * MH + mm
                                for tt in range(4):
                                    nc.tensor.matmul(
                                        outp[tt][:],
                                        gt[:, m, tt * 128:(tt + 1) * 128],
                                        w2t[:, mm, :],
                                        start=(m == 0), stop=(m == NMI - 1))
                        for tt in range(4):
                            os_ = top.tile([128, HSW], F32, tag="os")
                            nc.vector.tensor_add(os_[:], outp[tt][:],
                                                 x2ss[tt][:])
                            nc.sync.dma_start(
                                out[tt * 128:(tt + 1) * 128,
                                    hs * HSW:(hs + 1) * HSW], os_[:])

    return nc


# ---------------------------------------------------------------- host side
def _rope_tables(pos):
    inv = 1.0 / (THETA ** (np.arange(0, RD, 2, dtype=np.float32) / RD))
    f = pos[:, None].astype(np.float32) * inv[None, :]
    emb = np.concatenate([f, f], axis=-1)          # [T, RD]
    cos = np.ascontiguousarray(np.cos(emb).T)      # [RD, T]
    sin = np.sin(emb).T
    sinS = sin.copy()
    sinS[0:32] = -sin[0:32]
    return cos.astype(np.float32), np.ascontiguousarray(sinS).astype(np.float32)


def _band_masks():
    """Triangular masks for the 4 partial tiles of the banded attention:
    u=0: j<=p, u=1: j<=p+128, u=4: j>=p, u=5: j>=p+128."""
    import ml_dtypes
    p = np.arange(128)[:, None]
    j = np.arange(256)[None, :]
    m = np.stack([(j <= p), (j <= p + 128), (j >= p), (j >= p + 128)],
                 axis=1).astype(np.float32)
    return np.ascontiguousarray(m.astype(ml_dtypes.bfloat16))


def _prepare_in_maps(hidden_states, wq, wk, wv, wo, q_norm_w, k_norm_w,
                     ln1_w, ln2_w, w1, w2, w3):
    import ml_dtypes
    bf = ml_dtypes.bfloat16
    xf = np.ascontiguousarray(hidden_states.reshape(B * S, HID))
    wqTn = np.ascontiguousarray(wq.T.astype(bf))
    wkTn = np.ascontiguousarray(wk.T.astype(bf))
    wvTn = np.ascontiguousarray(wv.T.astype(bf))
    woTn = np.ascontiguousarray(wo.T.astype(bf))
    w1Pn = np.ascontiguousarray(
        w1.T.reshape(NM, 128, NCH1, CW).transpose(2, 0, 1, 3).astype(bf))
    w3Pn = np.ascontiguousarray(
        w3.T.reshape(NM, 128, NCH1, CW).transpose(2, 0, 1, 3).astype(bf))
    w2Pn = np.ascontiguousarray(
        w2.T.reshape(NMI, 128, HS, HSW).transpose(2, 0, 1, 3).astype(bf))
    ln1c = np.ascontiguousarray(ln1_w.reshape(16, 128).T)
    ln2c = np.ascontiguousarray(ln2_w.reshape(16, 128).T)
    qnc = np.ascontiguousarray(q_norm_w.reshape(16, 128).T)
    knc = np.ascontiguousarray(k_norm_w.reshape(4, 128).T)
    bandm = _band_masks()
    ones_c = np.ones((128, 1), np.float32)
    ones_r = np.ones((1, 128), np.float32)
    ones_b = np.ones((128, 1), ml_dtypes.bfloat16)

    in_maps = []
    for c in range(NCORES):
        t0 = c * TOK
        bidx = t0 // S
        s0 = t0 % S
        xe = np.zeros((EXT, HID), np.float32)
        lo = s0 - WIN
        if lo >= 0:
            xe[:] = xf[bidx * S + lo: bidx * S + s0 + TOK]
            halo_valid = True
        else:
            xe[WIN:] = xf[bidx * S + s0: bidx * S + s0 + TOK]
            halo_valid = False
        xTc = np.ascontiguousarray(xe.T)

        qpos = np.arange(s0, s0 + TOK)
        kpos = np.arange(s0 - WIN, s0 + TOK)
        cq, sq_ = _rope_tables(qpos)
        ck, sk_ = _rope_tables(np.maximum(kpos, 0))
        halo_bias = np.zeros(EXT, np.float32)
        if not halo_valid:
            halo_bias[0:WIN] = NEG
        haloc = np.ascontiguousarray(halo_bias.reshape(8, 128).T)

        in_maps.append({
            "xT": xTc,
            "wqT": wqTn, "wkT": wkTn, "wvT": wvTn, "woT": woTn,
            "w1P": w1Pn, "w3P": w3Pn, "w2P": w2Pn,
            "ln1w": ln1c, "ln2w": ln2c, "qnw": qnc, "knw": knc,
            "cos_q": cq, "sinS_q": sq_, "cos_k": ck, "sinS_k": sk_,
            "halo": haloc, "bandm": bandm,
            "ones_c": ones_c, "ones_r": ones_r, "ones_b": ones_b,
        })
    return in_maps


_NC = None


def _get_nc():
    global _NC
    if _NC is None:
        _register_ntff_hook()
        _NC = build_nc()
    return _NC


def run(in_maps, trace=False):
    from concourse.bass_utils import run_bass_kernel_spmd
    nc = _get_nc()
    return run_bass_kernel_spmd(nc, in_maps, core_ids=list(range(NCORES)),
                                trace=trace)


def kernel(**inputs):
    arrs = {k: np.asarray(v, dtype=np.float32) for k, v in inputs.items()}
    in_maps = _prepare_in_maps(
        arrs["hidden_states"], arrs["wq"], arrs["wk"], arrs["wv"], arrs["wo"],
        arrs["q_norm_w"], arrs["k_norm_w"], arrs["ln1_w"], arrs["ln2_w"],
        arrs["w1"], arrs["w2"], arrs["w3"])
    res = run(in_maps, trace=False)
    full = np.empty((B * S, HID), np.float32)
    for c in range(NCORES):
        full[c * TOK:(c + 1) * TOK] = res.results[c]["out"]
    return full.reshape(B, S, HID)


# revision 37
# speedup vs baseline: 1.2959x; 1.0929x over previous
"""MiniMax-M2 decoder layer on 8 trn2 NeuronCores (v2).

Sharding: fully sequence-sharded. Each core owns 512 tokens of the
flattened (B*S)=4096 token stream and recomputes the 512-token KV halo
locally (no collectives in the attention block). The MLP is
data-parallel over tokens too: every core streams the full bf16
w1/w3/w2 from its DRAM once (~100MB, hidden under the 655us of MLP
matmul) while computing only its own 512 tokens -> ZERO collectives in
the whole kernel.

All weights are pre-cast to bf16 and pre-packed on the host so every
weight DMA is a large contiguous block (128KB-512KB, 1-4KB per
partition line).

Matmuls run in bf16 against f32 psum. Softmax skips the
max-subtraction (scores are bounded ~6 for this layer's magnitudes).

Self-contained: includes the BIR wait-splitting fix this container's
walrus build needs (1 semaphore wait per instruction max).
"""

import json
import sys
import types

import numpy as np

import concourse.bass as bass
import concourse.mybir as mybir
import concourse.tile as tile
from concourse.masks import make_identity

# ---------------------------------------------------------------- constants
B, S, HID = 2, 2048, 2048
H, HK, D = 16, 4, 128
RD = 64
IM = 8192
WIN = 512
EPS = 1e-6
THETA = 10000.0
SCALE = D ** -0.5

NCORES = 8
TOK = 512              # own tokens per core
EXT = 1024             # halo + own
NEG = -1e9

F32 = mybir.dt.float32
F32R = mybir.dt.float32r
BF16 = mybir.dt.bfloat16
AF = mybir.ActivationFunctionType

KT = 8                 # 128-wide key tiles over EXT
NM = HID // 128        # 16 hid tiles
CW = 512               # im chunk width for w1/w3 streaming
NCH1 = IM // CW        # 16 chunks
NMI = IM // 128        # 64 im tiles
HS = 4                 # hid column quarters for w2 streaming
HSW = HID // HS        # 512

# ------------------------------------------------------- walrus wait-split fix
MAX_WAITS = 1


def _split_excess_waits(bir_bytes: bytes) -> bytes:
    m = json.loads(bir_bytes)
    ctr = [0]

    def fix_insts(insts):
        out = []
        for ins in insts:
            si = ins.get("sync_info")
            ow = (si or {}).get("on_wait") or []
            if len(ow) > MAX_WAITS:
                eng = ins["engine"]
                keep = ow[-MAX_WAITS:]
                excess = ow[:-MAX_WAITS]
                ins["sync_info"]["on_wait"] = keep
                for i in range(0, len(excess), MAX_WAITS):
                    ctr[0] += 1
                    out.append({
                        "debug": ins.get("debug", 0),
                        "engine": eng,
                        "ins": [],
                        "name": f"I-waitfix-{ctr[0]}",
                        "opcode": "NoOp",
                        "outs": [],
                        "sync_info": {"on_update": [],
                                      "on_wait": excess[i:i + MAX_WAITS]},
                        "text_hint": "waitfix",
                    })
            out.append(ins)
        return out

    def walk(o):
        if isinstance(o, dict):
            if isinstance(o.get("instructions"), list):
                o["instructions"] = fix_insts(o["instructions"])
            for v in o.values():
                walk(v)
        elif isinstance(o, list):
            for v in o:
                walk(v)

    walk(m)
    return json.dumps(m).encode()


class _BassFixed(bass.Bass):
    def to_json_bytes(self) -> bytes:
        return _split_excess_waits(super().to_json_bytes())


def _register_ntff_hook():
    """Provide antenv.axon_hooks (missing in this image) so trace=True works."""
    if "antenv.axon_hooks" in sys.modules:
        return
    try:
        import trn_agent_boot.trn_boot as tb
    except ImportError:
        return
    mod = types.ModuleType("antenv.axon_hooks")
    holder = [None]
    mod.set_axon_ntff_profile_hook = lambda h: holder.__setitem__(0, h)
    mod.get_axon_ntff_profile_hook = lambda: holder[0]
    sys.modules["antenv.axon_hooks"] = mod
    try:
        mod.set_axon_ntff_profile_hook(
            tb._ntff_profile_via_ctypes("/opt/axon/libaxon_pjrt.so"))
    except Exception:
        pass


# ---------------------------------------------------------------- the program
def build_nc():
    nc = _BassFixed(num_devices=NCORES, target_bir_lowering=False)

    xT = nc.dram_tensor("xT", [HID, EXT], F32R, kind="ExternalInput")
    wqT = nc.dram_tensor("wqT", [HID, H * D], BF16, kind="ExternalInput")
    wkT = nc.dram_tensor("wkT", [HID, HK * D], BF16, kind="ExternalInput")
    wvT = nc.dram_tensor("wvT", [HID, HK * D], BF16, kind="ExternalInput")
    woT = nc.dram_tensor("woT", [H * D, HID], BF16, kind="ExternalInput")
    w1P = nc.dram_tensor("w1P", [NCH1, NM, 128, CW], BF16,
                         kind="ExternalInput")
    w3P = nc.dram_tensor("w3P", [NCH1, NM, 128, CW], BF16,
                         kind="ExternalInput")
    w2P = nc.dram_tensor("w2P", [HS, NMI, 128, HSW], BF16,
                         kind="ExternalInput")
    ln1w = nc.dram_tensor("ln1w", [128, 16], F32, kind="ExternalInput")
    ln2w = nc.dram_tensor("ln2w", [128, 16], F32, kind="ExternalInput")
    qnw = nc.dram_tensor("qnw", [128, 16], F32, kind="ExternalInput")
    knw = nc.dram_tensor("knw", [128, 4], F32, kind="ExternalInput")
    cos_q = nc.dram_tensor("cos_q", [RD, TOK], F32, kind="ExternalInput")
    sinS_q = nc.dram_tensor("sinS_q", [RD, TOK], F32, kind="ExternalInput")
    cos_k = nc.dram_tensor("cos_k", [RD, EXT], F32, kind="ExternalInput")
    sinS_k = nc.dram_tensor("sinS_k", [RD, EXT], F32, kind="ExternalInput")
    halo = nc.dram_tensor("halo", [128, 8], F32, kind="ExternalInput")
    bandm = nc.dram_tensor("bandm", [128, 4, 256], BF16, kind="ExternalInput")
    ones_c = nc.dram_tensor("ones_c", [128, 1], F32R, kind="ExternalInput")
    ones_r = nc.dram_tensor("ones_r", [1, 128], F32R, kind="ExternalInput")
    ones_b = nc.dram_tensor("ones_b", [128, 1], BF16, kind="ExternalInput")

    out = nc.dram_tensor("out", [TOK, HID], F32, kind="ExternalOutput")

    with tile.TileContext(nc) as tc:
        with tc.tile_pool(name="consts", bufs=1) as cst, \
             tc.tile_pool(name="smalls", bufs=2) as sml, \
             tc.tile_pool(name="h2keep", bufs=1) as h2keep, \
             tc.tile_pool(name="dram", bufs=1, space="DRAM") as dram:

            # ---------------- constants
            onec = cst.tile([128, 1], F32R)
            nc.sync.dma_start(onec[:], ones_c[:])
            oner = cst.tile([1, 128], F32R)
            nc.sync.dma_start(oner[:], ones_r[:])
            oneb = cst.tile([128, 1], BF16)
            nc.sync.dma_start(oneb[:], ones_b[:])
            ln1w_s = cst.tile([128, 16], F32)
            nc.sync.dma_start(ln1w_s[:], ln1w[:])
            ln2w_s = cst.tile([128, 16], F32)
            nc.sync.dma_start(ln2w_s[:], ln2w[:])
            qnw_s = cst.tile([128, 16], F32)
            nc.sync.dma_start(qnw_s[:], qnw[:])
            knw_s = cst.tile([128, 4], F32)
            nc.sync.dma_start(knw_s[:], knw[:])
            eps_s = cst.tile([1, 1], F32)
            nc.vector.memset(eps_s[:], EPS)
            identF = cst.tile([128, 128], F32)
            make_identity(nc, identF[:])
            identR = cst.tile([128, 128], F32R)
            nc.vector.tensor_copy(identR[:], identF[:])

            h2s = h2keep.tile([128, NM, TOK], BF16)
            x2tok = dram.tile([TOK, HID], F32)

            with tc.tile_pool(name="rowps", bufs=3, space="PSUM") as rps, \
                 tc.tile_pool(name="bigps", bufs=4, space="PSUM") as bps, \
                 tc.tile_pool(name="bcps", bufs=1, space="PSUM") as bcps:

                def bcast_row(row_f32r, width, tag):
                    """[1,width] F32R row -> [128,width] F32 psum broadcast."""
                    p = bcps.tile([128, width], F32, tag="bc")
                    for j in range(0, width, 512):
                        w = min(512, width - j)
                        nc.tensor.matmul(p[:, j:j + w], oner[:],
                                         row_f32r[:, j:j + w],
                                         start=True, stop=True)
                    return p

                def row_rsqrt_bcast(acc_parts, width, denom, tag):
                    """psum [1,*] sum-of-squares parts -> [128,width] psum with
                    1/sqrt(acc/denom + eps) broadcast over partitions.
                    rsqrt(x) = exp(-0.5*ln(x)) on ScalarE (DVE reciprocal on a
                    [1,w] row is ~3.4us: one lane, Newton iterations)."""
                    srow = sml.tile([1, width], F32, tag="srow")
                    for (p, j, w) in acc_parts:
                        nc.scalar.activation(out=srow[:, j:j + w], in_=p[:],
                                             func=AF.Ln, bias=eps_s[:],
                                             scale=1.0 / denom)
                    rrow = sml.tile([1, width], F32R, tag="rrow")
                    nc.scalar.activation(out=rrow[:], in_=srow[:],
                                         func=AF.Exp, scale=-0.5)
                    return bcast_row(rrow, width, tag)

                # ======== q/k/v activations
                with tc.tile_pool(name="qkv", bufs=1) as qkv:
                    qT = qkv.tile([128, H, TOK], BF16)
                    kT = qkv.tile([128, HK, EXT], BF16)
                    Vb = qkv.tile([128, KT, HK * D], BF16)

                    # ============ phase A+B: ln1-RMS + Q/K/V projections
                    with tc.tile_pool(name="hp", bufs=1) as hp, \
                         tc.tile_pool(name="xs", bufs=4) as xs, \
                         tc.tile_pool(name="sqp", bufs=2) as sqp, \
                         tc.tile_pool(name="wat", bufs=1) as wat:
                        vT = hp.tile([128, HK, EXT], F32R, tag="vT")
                        hTh = hp.tile([128, NM, EXT], BF16, tag="h")

                        # resident attention weights (big contiguous strips)
                        wks = wat.tile([128, NM, HK * D], BF16)
                        wvs = wat.tile([128, NM, HK * D], BF16)
                        nc.sync.dma_start(
                            wks[:], wkT[:].rearrange("(i p) j -> p i j", p=128))
                        nc.sync.dma_start(
                            wvs[:], wvT[:].rearrange("(i p) j -> p i j", p=128))

                        for half in (0, 1):
                            c0 = half * 512
                            acc = rps.tile([1, 512], F32, tag="row")
                            for i in range(NM):
                                xt = xs.tile([128, 512], F32R, tag="xt")
                                nc.scalar.dma_start(
                                    xt[:],
                                    xT[i * 128:(i + 1) * 128, c0:c0 + 512])
                                sq = sqp.tile([128, 512], F32R, tag="sq")
                                nc.vector.tensor_mul(sq[:], xt[:], xt[:])
                                nc.tensor.matmul(acc[:], onec[:], sq[:],
                                                 start=(i == 0),
                                                 stop=(i == NM - 1))
                            s1b = row_rsqrt_bcast([(acc, 0, 512)], 512,
                                                  HID, "l1")
                            for i in range(NM):
                                xt = xs.tile([128, 512], F32R, tag="xt")
                                nc.scalar.dma_start(
                                    xt[:],
                                    xT[i * 128:(i + 1) * 128, c0:c0 + 512])
                                tmp = sqp.tile([128, 512], F32R, tag="sq")
                                nc.vector.tensor_mul(tmp[:], xt[:], s1b[:])
                                nc.vector.tensor_scalar_mul(
                                    hTh[:, i, c0:c0 + 512], tmp[:],
                                    ln1w_s[:, i:i + 1])

                            # K / V for this half
                            for m in range(HK):
                                pk = bps.tile([128, 512], F32, tag="big")
                                for i in range(NM):
                                    nc.tensor.matmul(
                                        pk[:], wks[:, i, m * 128:(m + 1) * 128],
                                        hTh[:, i, c0:c0 + 512],
                                        start=(i == 0), stop=(i == NM - 1))
                                pv = bps.tile([128, 512], F32, tag="big")
                                for i in range(NM):
                                    nc.tensor.matmul(
                                        pv[:], wvs[:, i, m * 128:(m + 1) * 128],
                                        hTh[:, i, c0:c0 + 512],
                                        start=(i == 0), stop=(i == NM - 1))
                                nc.vector.tensor_copy(kT[:, m, c0:c0 + 512],
                                                      pk[:])
                                nc.vector.tensor_copy(vT[:, m, c0:c0 + 512],
                                                      pv[:])

                        # Q projection (own tokens only), wq in two halves
                        for qh in range(2):
                            wqs = wat.tile([128, NM, H * D // 2], BF16,
                                           tag="wq", bufs=1,
                                           name=f"wqs{qh}")
                            nc.sync.dma_start(
                                wqs[:],
                                wqT[:, qh * 1024:(qh + 1) * 1024]
                                .rearrange("(i p) j -> p i j", p=128))
                            for mm in range(H // 2):
                                m = qh * 8 + mm
                                pq = bps.tile([128, 512], F32, tag="big")
                                for i in range(NM):
                                    nc.tensor.matmul(
                                        pq[:],
                                        wqs[:, i, mm * 128:(mm + 1) * 128],
                                        hTh[:, i, 512:1024],
                                        start=(i == 0), stop=(i == NM - 1))
                                nc.vector.tensor_copy(qT[:, m, :], pq[:])

                        # transpose vT -> token-major bf16 Vb
                        for kt in range(KT):
                            for g in range(HK):
                                pt = bps.tile([128, 128], F32R, tag="big")
                                nc.tensor.transpose(
                                    pt[:], vT[:, g, kt * 128:(kt + 1) * 128],
                                    identR[:])
                                nc.vector.tensor_copy(
                                    Vb[:, kt, g * 128:(g + 1) * 128], pt[:])

                    # ---------------- fused q/k RMSNorm + partial RoPE
                    with tc.tile_pool(name="nrm", bufs=1) as nrm:
                        cq_s = nrm.tile([RD, TOK], F32)
                        nc.sync.dma_start(cq_s[:], cos_q[:])
                        sq_s = nrm.tile([RD, TOK], F32)
                        nc.sync.dma_start(sq_s[:], sinS_q[:])
                        ck_s = nrm.tile([RD, EXT], F32)
                        nc.sync.dma_start(ck_s[:], cos_k[:])
                        sk_s = nrm.tile([RD, EXT], F32)
                        nc.sync.dma_start(sk_s[:], sinS_k[:])

                        accq = rps.tile([1, 512], F32, tag="row")
                        sqq = nrm.tile([128, TOK], BF16, tag="nsq")
                        for h in range(H):
                            nc.vector.tensor_mul(sqq[:], qT[:, h, :],
                                                 qT[:, h, :])
                            nc.tensor.matmul(accq[:], oneb[:], sqq[:],
                                             start=(h == 0), stop=(h == H - 1))
                        cqb = row_rsqrt_bcast([(accq, 0, 512)], TOK,
                                              H * D, "qn")
                        cqb3 = cqb[:].rearrange("p (g t) -> p g t",
                                                g=1).broadcast_to(
                                                    [128, H, TOK])
                        nc.vector.tensor_mul(qT[:], qT[:], cqb3)
                        for h in range(H):
                            nc.vector.tensor_scalar_mul(
                                qT[:, h, :], qT[:, h, :], qnw_s[:, h:h + 1])

                        acck_lo = rps.tile([1, 512], F32, tag="row")
                        acck_hi = rps.tile([1, 512], F32, tag="row")
                        sqk = nrm.tile([128, EXT], BF16, tag="nsqk")
                        for g in range(HK):
                            nc.vector.tensor_mul(sqk[:], kT[:, g, :],
                                                 kT[:, g, :])
                            nc.tensor.matmul(acck_lo[:], oneb[:],
                                             sqk[:, 0:512],
                                             start=(g == 0), stop=(g == HK - 1))
                            nc.tensor.matmul(acck_hi[:], oneb[:],
                                             sqk[:, 512:1024],
                                             start=(g == 0), stop=(g == HK - 1))
                        ckb_lo = row_rsqrt_bcast([(acck_lo, 0, 512)], 512,
                                                 HK * D, "kn")
                        c3lo = ckb_lo[:].rearrange(
                            "p (g t) -> p g t", g=1).broadcast_to(
                            [128, HK, 512])
                        nc.vector.tensor_mul(kT[:, :, 0:512],
                                             kT[:, :, 0:512], c3lo)
                        ckb_hi = row_rsqrt_bcast([(acck_hi, 0, 512)], 512,
                                                 HK * D, "kn2")
                        c3hi = ckb_hi[:].rearrange(
                            "p (g t) -> p g t", g=1).broadcast_to(
                            [128, HK, 512])
                        nc.vector.tensor_mul(kT[:, :, 512:1024],
                                             kT[:, :, 512:1024], c3hi)
                        for g in range(HK):
                            nc.vector.tensor_scalar_mul(
                                kT[:, g, :], kT[:, g, :], knw_s[:, g:g + 1])

                        def rope(t3, nh, width, cos_t, sinS_t):
                            c3 = cos_t[:].rearrange(
                                "p (g t) -> p g t", g=1).broadcast_to(
                                [RD, nh, width])
                            s3 = sinS_t[:].rearrange(
                                "p (g t) -> p g t", g=1).broadcast_to(
                                [RD, nh, width])
                            # partner halves moved onto matching partitions
                            qsw = nrm.tile([RD, nh, width], BF16, tag="rsw")
                            nc.sync.dma_start(qsw[0:32], t3[32:64])
                            nc.sync.dma_start(qsw[32:64], t3[0:32])
                            t1 = nrm.tile([RD, nh, width], BF16, tag="rt1")
                            nc.vector.tensor_mul(t1[:], t3[0:RD], c3)
                            nc.vector.tensor_mul(qsw[:], qsw[:], s3)
                            nc.vector.tensor_add(t3[0:RD], t1[:], qsw[:])

                        rope(qT[:, 0:8, :], 8, TOK, cq_s, sq_s)
                        rope(qT[:, 8:16, :], 8, TOK, cq_s, sq_s)
                        rope(kT[:], HK, EXT, ck_s, sk_s)

                    # ============ phase C: sliding-window attention
                    with tc.tile_pool(name="attn", bufs=1) as ap, \
                         tc.tile_pool(name="es", bufs=16) as es, \
                         tc.tile_pool(name="wop", bufs=2) as wop:
                        # prefetch first o_proj weight quarter during attention
                        NOH = 4
                        OHW = HID // NOH
                        wohs = [None] * NOH

                        def load_woh(oh):
                            t = wop.tile([128, NM, OHW], BF16, tag="wo",
                                         name=f"woh{oh}")
                            nc.sync.dma_start(
                                t[:], woT[:, oh * OHW:(oh + 1) * OHW]
                                .rearrange("(i p) j -> p i j", p=128))
                            wohs[oh] = t

                        load_woh(0)
                        halo_s = ap.tile([128, 8], F32)
                        nc.sync.dma_start(halo_s[:], halo[:])
                        bandm_s = ap.tile([128, 4, 256], BF16)
                        nc.sync.dma_start(bandm_s[:], bandm[:])
                        attnT = ap.tile([128, H, TOK], BF16)
                        # banded: query chunks of 256 see only 6 key tiles
                        MIDX = {0: 0, 1: 1, 4: 2, 5: 3}
                        for h in range(H):
                            g = h // (H // HK)
                            for j2 in range(2):
                                q0 = j2 * 256
                                es_l = []
                                for u in range(6):
                                    kt = 2 * j2 + u
                                    ps = bps.tile([128, 256], F32, tag="big",
                                                  name=f"ps{h}_{j2}_{u}")
                                    nc.tensor.matmul(
                                        ps[:],
                                        kT[:, g, kt * 128:(kt + 1) * 128],
                                        qT[:, h, q0:q0 + 256],
                                        start=True, stop=True)
                                    e = es.tile([128, 256], BF16, tag="e",
                                                name=f"e{h}_{j2}_{u}")
                                    nc.scalar.activation(
                                        out=e[:], in_=ps[:], func=AF.Exp,
                                        bias=halo_s[:, kt:kt + 1], scale=SCALE)
                                    if u in MIDX:
                                        nc.vector.tensor_mul(
                                            e[:], e[:],
                                            bandm_s[:, MIDX[u], :])
                                    es_l.append(e)
                                # elementwise sum over the 6 key tiles, then
                                # one matmul reduces over the 128 partitions
                                t1 = es.tile([128, 256], BF16, tag="t",
                                             name=f"t1_{h}_{j2}")
                                nc.vector.tensor_add(t1[:], es_l[0][:],
                                                     es_l[1][:])
                                t2 = es.tile([128, 256], BF16, tag="t",
                                             name=f"t2_{h}_{j2}")
                                nc.vector.tensor_add(t2[:], es_l[2][:],
                                                     es_l[3][:])
                                t3 = es.tile([128, 256], BF16, tag="t",
                                             name=f"t3_{h}_{j2}")
                                nc.vector.tensor_add(t3[:], es_l[4][:],
                                                     es_l[5][:])
                                t4 = es.tile([128, 256], BF16, tag="t",
                                             name=f"t4_{h}_{j2}")
                                nc.vector.tensor_add(t4[:], t1[:], t2[:])
                                t5 = es.tile([128, 256], BF16, tag="t",
                                             name=f"t5_{h}_{j2}")
                                nc.vector.tensor_add(t5[:], t4[:], t3[:])
                                den = rps.tile([1, 256], F32, tag="row",
                                               name=f"den{h}_{j2}")
                                nc.tensor.matmul(den[:], oneb[:], t5[:],
                                                 start=True, stop=True)
                                po = bps.tile([128, 256], F32, tag="big",
                                              name=f"po{h}_{j2}")
                                for idx, e in enumerate(es_l):
                                    kt = 2 * j2 + idx
                                    nc.tensor.matmul(
                                        po[:],
                                        Vb[:, kt, g * 128:(g + 1) * 128],
                                        e[:], start=(idx == 0),
                                        stop=(idx == 5))
                                dln = sml.tile([1, 256], F32, tag="dln",
                                               name=f"dln{h}_{j2}")
                                nc.scalar.activation(out=dln[:], in_=den[:],
                                                     func=AF.Ln)
                                drr = sml.tile([1, 256], F32R, tag="drr",
                                               name=f"drr{h}_{j2}")
                                nc.scalar.activation(out=drr[:], in_=dln[:],
                                                     func=AF.Exp, scale=-1.0)
                                rb = bcast_row(drr, 256, "rden")
                                rbs = sml.tile([128, 256], F32R, tag="rbs",
                                               name=f"rbs{h}_{j2}")
                                nc.vector.tensor_copy(rbs[:], rb[:])
                                nc.vector.tensor_mul(
                                    attnT[:, h, q0:q0 + 256], po[:], rbs[:])

                        # ============ phase D: o_proj + residual + ln2
                        with tc.tile_pool(name="x2", bufs=1) as x2p, \
                             tc.tile_pool(name="xs2", bufs=3) as xs2:
                            x2T = x2p.tile([128, NM, TOK], F32)
                            acc2 = rps.tile([1, 512], F32, tag="row")
                            for m in range(NM):
                                oh, mo = divmod(m, NM // NOH)
                                if mo == 0 and oh + 1 < NOH and \
                                        wohs[oh + 1] is None:
                                    load_woh(oh + 1)
                                px = bps.tile([128, 512], F32, tag="big")
                                for i in range(NM):
                                    nc.tensor.matmul(
                                        px[:],
                                        wohs[oh][:, i, mo * 128:(mo + 1) * 128],
                                        attnT[:, i, :],
                                        start=(i == 0), stop=(i == NM - 1))
                                xo = xs2.tile([128, TOK], F32R, tag="xo")
                                nc.scalar.dma_start(
                                    xo[:], xT[m * 128:(m + 1) * 128, 512:1024])
                                nc.vector.tensor_add(x2T[:, m, :], px[:], xo[:])
                                sq2 = xs2.tile([128, TOK], F32R, tag="sq2")
                                nc.vector.tensor_mul(sq2[:], x2T[:, m, :],
                                                     x2T[:, m, :])
                                nc.tensor.matmul(acc2[:], onec[:], sq2[:],
                                                 start=(m == 0),
                                                 stop=(m == NM - 1))

                            # x2 token-major -> DRAM (for final residual)
                            for tt in range(4):
                                for grp in range(4):
                                    ts = xs2.tile([128, 512], F32, tag="x2t")
                                    for j in range(4):
                                        m = grp * 4 + j
                                        pt = bps.tile([128, 128], F32,
                                                      tag="big")
                                        nc.tensor.transpose(
                                            pt[:],
                                            x2T[:, m, tt * 128:(tt + 1) * 128],
                                            identF[:])
                                        nc.vector.tensor_copy(
                                            ts[:, j * 128:(j + 1) * 128],
                                            pt[:])
                                    nc.scalar.dma_start(
                                        x2tok[tt * 128:(tt + 1) * 128,
                                              grp * 512:(grp + 1) * 512],
                                        ts[:])

                            s2b = row_rsqrt_bcast([(acc2, 0, 512)], TOK,
                                                  HID, "l2")
                            for m in range(NM):
                                h2t = xs2.tile([128, TOK], BF16, tag="h2t")
                                nc.vector.tensor_mul(h2t[:], x2T[:, m, :],
                                                     s2b[:])
                                nc.vector.tensor_scalar_mul(
                                    h2s[:, m, :], h2t[:], ln2w_s[:, m:m + 1])

            # ============ phase E: data-parallel MLP, streamed bf16 weights
            with tc.tile_pool(name="gpool", bufs=1) as gpo:
                gt = gpo.tile([128, NMI, TOK], BF16)

                with tc.tile_pool(name="slab", bufs=2) as slp, \
                     tc.tile_pool(name="silp", bufs=3) as silp, \
                     tc.tile_pool(name="eps", bufs=4, space="PSUM") as eps:
                    for c in range(NCH1):
                        w1s = slp.tile([128, NM, CW], BF16, tag="w1s")
                        w3s = slp.tile([128, NM, CW], BF16, tag="w3s")
                        nc.sync.dma_start(
                            w1s[:], w1P[c].rearrange("i p j -> p i j"))
                        nc.sync.dma_start(
                            w3s[:], w3P[c].rearrange("i p j -> p i j"))
                        for mm in range(CW // 128):
                            mi = c * (CW // 128) + mm
                            pa = eps.tile([128, 512], F32, tag="e")
                            for i in range(NM):
                                nc.tensor.matmul(
                                    pa[:], w1s[:, i, mm * 128:(mm + 1) * 128],
                                    h2s[:, i, :],
                                    start=(i == 0), stop=(i == NM - 1))
                            pb = eps.tile([128, 512], F32, tag="e")
                            for i in range(NM):
                                nc.tensor.matmul(
                                    pb[:], w3s[:, i, mm * 128:(mm + 1) * 128],
                                    h2s[:, i, :],
                                    start=(i == 0), stop=(i == NM - 1))
                            sil = silp.tile([128, 512], F32, tag="sil")
                            nc.scalar.activation(out=sil[:], in_=pa[:],
                                                 func=AF.Silu)
                            nc.vector.tensor_mul(gt[:, mi, :], sil[:], pb[:])

                # pass 2: out = g @ w2 + x2   (w2 streamed once)
                MH = NMI // 2   # 32 im strips per w2 slab
                with tc.tile_pool(name="w2s", bufs=2) as w2sp, \
                     tc.tile_pool(name="ops", bufs=4, space="PSUM") as ops, \
                     tc.tile_pool(name="top", bufs=4) as top:
                    for hs in range(HS):
                        outp = [ops.tile([128, HSW], F32, tag="o",
                                         name=f"outp{hs}_{tt}")
                                for tt in range(4)]
                        x2ss = []
                        for tt in range(4):
                            x2s = top.tile([128, HSW], F32, tag="x2s",
                                           name=f"x2s{hs}_{tt}")
                            nc.sync.dma_start(
                                x2s[:], x2tok[tt * 128:(tt + 1) * 128,
                                              hs * HSW:(hs + 1) * HSW])
                            x2ss.append(x2s)
                        for mh in range(2):
                            w2t = w2sp.tile([128, MH, HSW], BF16, tag="w2t")
                            nc.sync.dma_start(
                                w2t[:], w2P[hs, mh * MH:(mh + 1) * MH]
                                .rearrange("m p j -> p m j"))
                            for mm in range(MH):
                                m = mh * MH + mm
                                for tt in range(4):
                                    nc.tensor.matmul(
                                        outp[tt][:],
                                        gt[:, m, tt * 128:(tt + 1) * 128],
                                        w2t[:, mm, :],
                                        start=(m == 0), stop=(m == NMI - 1))
                        for tt in range(4):
                            os_ = top.tile([128, HSW], F32, tag="os")
                            nc.vector.tensor_add(os_[:], outp[tt][:],
                                                 x2ss[tt][:])
                            nc.sync.dma_start(
                                out[tt * 128:(tt + 1) * 128,
                                    hs * HSW:(hs + 1) * HSW], os_[:])

    return nc


# ---------------------------------------------------------------- host side
def _rope_tables(pos):
    inv = 1.0 / (THETA ** (np.arange(0, RD, 2, dtype=np.float32) / RD))
    f = pos[:, None].astype(np.float32) * inv[None, :]
    emb = np.concatenate([f, f], axis=-1)          # [T, RD]
    cos = np.ascontiguousarray(np.cos(emb).T)      # [RD, T]
    sin = np.sin(emb).T
    sinS = sin.copy()
    sinS[0:32] = -sin[0:32]
    return cos.astype(np.float32), np.ascontiguousarray(sinS).astype(np.float32)


def _band_masks():
    """Triangular masks for the 4 partial tiles of the banded attention:
    u=0: j<=p, u=1: j<=p+128, u=4: j>=p, u=5: j>=p+128."""
    import ml_dtypes
    p = np.arange(128)[:, None]
    j = np.arange(256)[None, :]
    m = np.stack([(j <= p), (j <= p + 128), (j >= p), (j >= p + 128)],
                 axis=1).astype(np.float32)
    return np.ascontiguousarray(m.astype(ml_dtypes.bfloat16))


def _prepare_in_maps(hidden_states, wq, wk, wv, wo, q_norm_w, k_norm_w,
                     ln1_w, ln2_w, w1, w2, w3):
    import ml_dtypes
    bf = ml_dtypes.bfloat16
    xf = np.ascontiguousarray(hidden_states.reshape(B * S, HID))
    wqTn = np.ascontiguousarray(wq.T.astype(bf))
    wkTn = np.ascontiguousarray(wk.T.astype(bf))
    wvTn = np.ascontiguousarray(wv.T.astype(bf))
    woTn = np.ascontiguousarray(wo.T.astype(bf))
    w1Pn = np.ascontiguousarray(
        w1.T.reshape(NM, 128, NCH1, CW).transpose(2, 0, 1, 3).astype(bf))
    w3Pn = np.ascontiguousarray(
        w3.T.reshape(NM, 128, NCH1, CW).transpose(2, 0, 1, 3).astype(bf))
    w2Pn = np.ascontiguousarray(
        w2.T.reshape(NMI, 128, HS, HSW).transpose(2, 0, 1, 3).astype(bf))
    ln1c = np.ascontiguousarray(ln1_w.reshape(16, 128).T)
    ln2c = np.ascontiguousarray(ln2_w.reshape(16, 128).T)
    qnc = np.ascontiguousarray(q_norm_w.reshape(16, 128).T)
    knc = np.ascontiguousarray(k_norm_w.reshape(4, 128).T)
    bandm = _band_masks()
    ones_c = np.ones((128, 1), np.float32)
    ones_r = np.ones((1, 128), np.float32)
    ones_b = np.ones((128, 1), ml_dtypes.bfloat16)

    in_maps = []
    for c in range(NCORES):
        t0 = c * TOK
        bidx = t0 // S
        s0 = t0 % S
        xe = np.zeros((EXT, HID), np.float32)
        lo = s0 - WIN
        if lo >= 0:
            xe[:] = xf[bidx * S + lo: bidx * S + s0 + TOK]
            halo_valid = True
        else:
            xe[WIN:] = xf[bidx * S + s0: bidx * S + s0 + TOK]
            halo_valid = False
        xTc = np.ascontiguousarray(xe.T)

        qpos = np.arange(s0, s0 + TOK)
        kpos = np.arange(s0 - WIN, s0 + TOK)
        cq, sq_ = _rope_tables(qpos)
        ck, sk_ = _rope_tables(np.maximum(kpos, 0))
        halo_bias = np.zeros(EXT, np.float32)
        if not halo_valid:
            halo_bias[0:WIN] = NEG
        haloc = np.ascontiguousarray(halo_bias.reshape(8, 128).T)

        in_maps.append({
            "xT": xTc,
            "wqT": wqTn, "wkT": wkTn, "wvT": wvTn, "woT": woTn,
            "w1P": w1Pn, "w3P": w3Pn, "w2P": w2Pn,
            "ln1w": ln1c, "ln2w": ln2c, "qnw": qnc, "knw": knc,
            "cos_q": cq, "sinS_q": sq_, "cos_k": ck, "sinS_k": sk_,
            "halo": haloc, "bandm": bandm,
            "ones_c": ones_c, "ones_r": ones_r, "ones_b": ones_b,
        })
    return in_maps


_NC = None


def _get_nc():
    global _NC
    if _NC is None:
        _register_ntff_hook()
        _NC = build_nc()
    return _NC


def run(in_maps, trace=False):
    from concourse.bass_utils import run_bass_kernel_spmd
    nc = _get_nc()
    return run_bass_kernel_spmd(nc, in_maps, core_ids=list(range(NCORES)),
                                trace=trace)


def kernel(**inputs):
    arrs = {k: np.asarray(v, dtype=np.float32) for k, v in inputs.items()}
    in_maps = _prepare_in_maps(
        arrs["hidden_states"], arrs["wq"], arrs["wk"], arrs["wv"], arrs["wo"],
        arrs["q_norm_w"], arrs["k_norm_w"], arrs["ln1_w"], arrs["ln2_w"],
        arrs["w1"], arrs["w2"], arrs["w3"])
    res = run(in_maps, trace=False)
    full = np.empty((B * S, HID), np.float32)
    for c in range(NCORES):
        full[c * TOK:(c + 1) * TOK] = res.results[c]["out"]
    return full.reshape(B, S, HID)
